# revision 1
# baseline (speedup 1.0000x reference)
"""Trainium2 Bass kernel for nn_AttnAutoEncoderRNN (H=1024, V=50257, T=256).

Strategy:
  - The GRU encoder/decoder recurrence is inherently sequential (batch=1), so
    it is replicated on all 8 cores (per-step cross-core sync is impossible:
    the on-device AllReduce floor ~10us exceeds a whole step).
  - The big [V,H] output projection + log_softmax is vocab-sharded across the
    8 cores; a single tiny AllReduce combines the per-shard exp-sums.
  - Heavy matmuls use bf16 operands with fp32 PSUM accumulation.
  - sigmoid(x) = 0.5*tanh(x/2)+0.5 keeps the whole recurrence in one ACT
    table set (exp/tanh/relu share a set; sigmoid does not).
  - Attention context folded: M2 = enc_outs @ C2.T is precomputed once, so a
    decoder step needs e @ M2 (a [T]-matvec) instead of ctx and C2 @ ctx.
  - Division-free softmax: comb = relu(P_C[t] + (e @ M2) * (1/S)).
"""

import numpy as np
import ml_dtypes

import concourse.bass as bass
import concourse.bacc as bacc
import concourse.tile as tile
import concourse.mybir as mybir
from concourse.bass_utils import run_bass_kernel_spmd

BF16 = ml_dtypes.bfloat16
F32 = mybir.dt.float32
BF = mybir.dt.bfloat16
AF = mybir.ActivationFunctionType
ALU = mybir.AluOpType

H = 1024
HC = H // 128            # 8 k-chunks of the hidden dim
G = 3 * H                # 3072 gate rows
GC = G // 128            # 24 gate m-tiles
V_FULL = 50257
N_CORES = 8
SOS = 1
NV = 512                 # vocab tile width in the projection


def _cdiv(a, b):
    return (a + b - 1) // b


def build_program(T, VS, VA):
    """T timesteps, VS = padded vocab shard, VA = part of VS loaded early."""
    SC = _cdiv(T, 128)             # chunks of the attention (T) axis
    TC = SC                        # time chunks (projection M-tiles)
    s_last = T - (SC - 1) * 128
    VC = _cdiv(VS, NV)
    GS = G + T                     # stacked [dec_Whh; A2] rows
    GSC = _cdiv(GS, 128)
    VCA = VA // NV                 # early vocab chunks (VA multiple of NV)

    nc = bacc.Bacc("TRN2", target_bir_lowering=False, debug=False,
                   num_devices=N_CORES)

    def din(name, shape, dt):
        return nc.dram_tensor(name, shape, dt, kind="ExternalInput").ap()

    wihe_t = din("wihe_t", [128, HC * G], BF)
    whhe_t = din("whhe_t", [128, HC * G], BF)
    wihd_t = din("wihd_t", [128, HC * G], BF)
    wstk_t = din("wstk_t", [128, HC * GS], BF)
    c1t = din("c1t", [128, HC * H], BF)
    c2t = din("c2t", [128, HC * H], BF)
    a1t = din("a1t", [128, HC * T], BF)
    embt = din("embt", [128, HC * T], BF)
    inpt = din("inpt", [128, HC * T], BF)
    bias_e = din("bias_e", [128, GC], F32)
    bhh_n_e = din("bhh_n_e", [128, HC], F32)
    biasd_rz = din("biasd_rz", [128, 16], F32)
    biasd_hn = din("biasd_hn", [128, HC], F32)
    biasd_xn = din("biasd_xn", [128, HC], F32)
    bias_a = din("bias_a", [128, SC], F32)
    bias_c = din("bias_c", [128, HC], F32)
    out_wta = din("out_wta", [128, HC * VA], BF)
    out_wtb = din("out_wtb", [128, HC * (VS - VA)], BF)
    out_bb = din("out_bb", [1, VS], BF)

    out_d = nc.dram_tensor("out", [T, VS], F32, kind="ExternalOutput").ap()
    dbg_enc = nc.dram_tensor("dbg_enc", [128, HC * T], BF,
                             kind="ExternalOutput").ap()
    dbg_hdec = nc.dram_tensor("dbg_hdec", [128, HC * T], BF,
                              kind="ExternalOutput").ap()

    with tile.TileContext(nc) as tc:
        # ----------------- persistent tiles -----------------
        cons_cm = tc.tile_pool(name="cons", bufs=1)
        cons = cons_cm.__enter__()
        enc_outsT = cons.tile([128, HC, T], BF, tag="enc_outsT")
        h_decT = cons.tile([128, HC, T], BF, tag="h_decT")
        m2_sb = cons.tile([128, TC, H], BF, tag="m2")
        pc_sb = cons.tile([128, HC, T], F32, tag="pc")
        pa_sb = cons.tile([128, SC, T], F32, tag="pa")
        be_sb = cons.tile([128, GC], F32, tag="be")
        bhn_e = cons.tile([128, HC], F32, tag="bhne")
        brz_d = cons.tile([128, 16], F32, tag="brzd")
        bhn_d = cons.tile([128, HC], F32, tag="bhnd")
        bxn_d = cons.tile([128, HC], F32, tag="bxnd")
        ba_sb = cons.tile([128, SC], F32, tag="ba")
        bc_sb = cons.tile([128, HC], F32, tag="bc")
        ones_col = cons.tile([128, 1], F32, tag="ones_col")
        ones_bf = cons.tile([1, 128], BF, tag="ones_bf")

        nc.sync.dma_start(be_sb[:], bias_e[:])
        nc.sync.dma_start(bhn_e[:], bhh_n_e[:])
        nc.sync.dma_start(brz_d[:], biasd_rz[:])
        nc.sync.dma_start(bhn_d[:], biasd_hn[:])
        nc.sync.dma_start(bxn_d[:], biasd_xn[:])
        nc.sync.dma_start(ba_sb[:], bias_a[:])
        nc.sync.dma_start(bc_sb[:], bias_c[:])
        nc.vector.memset(ones_col[:], 1.0)
        nc.vector.memset(ones_bf[:], 1.0)

        # fp32 h state + small per-step work tiles
        hpool_cm = tc.tile_pool(name="hpool", bufs=3)
        hpool = hpool_cm.__enter__()
        work_cm = tc.tile_pool(name="work", bufs=3)
        work = work_cm.__enter__()

        # enc-phase tensors (freed after M2)
        encw_cm = tc.tile_pool(name="encw", bufs=1)
        encw = encw_cm.__enter__()
        whhe = encw.tile([128, HC, G], BF, tag="whhe")
        gxp = encw.tile([128, GC, T], F32, tag="gxp")
        c2 = encw.tile([128, HC, H], BF, tag="c2")
        nc.sync.dma_start(whhe[:], whhe_t[:])
        nc.sync.dma_start(c2[:], c2t[:])

        # ----------------- precompute phase -----------------
        with tc.tile_pool(name="pre", bufs=1) as pre, \
             tc.tile_pool(name="prepsum", bufs=6, space="PSUM") as pps:
            wihe = pre.tile([128, HC, G], BF, tag="wihe")
            c1 = pre.tile([128, HC, H], BF, tag="c1")
            a1 = pre.tile([128, HC, T], BF, tag="a1")
            emb = pre.tile([128, HC, T], BF, tag="emb")
            inp = pre.tile([128, HC, T], BF, tag="inp")
            nc.sync.dma_start(wihe[:], wihe_t[:])
            nc.sync.dma_start(c1[:], c1t[:])
            nc.sync.dma_start(a1[:], a1t[:])
            nc.sync.dma_start(emb[:], embt[:])
            nc.sync.dma_start(inp[:], inpt[:])

            # gxp[:, gc, t] = (input_seq @ enc_Wih.T).T + folded enc biases
            for gc in range(GC):
                ps = pps.tile([128, T], F32, tag="pp")
                for kc in range(HC):
                    nc.tensor.matmul(
                        ps[:], wihe[:, kc, gc * 128:(gc + 1) * 128],
                        inp[:, kc, :], start=(kc == 0), stop=(kc == HC - 1),
                        skip_group_check=True)
                nc.vector.tensor_scalar(
                    out=gxp[:, gc, :], in0=ps[:],
                    scalar1=be_sb[:, gc:gc + 1], scalar2=None, op0=ALU.add)

            # pc_sb[:, mc, t] = (emb_seq @ C1.T).T + comb_b
            for mc in range(HC):
                ps = pps.tile([128, T], F32, tag="pp")
                for kc in range(HC):
                    nc.tensor.matmul(
                        ps[:], c1[:, kc, mc * 128:(mc + 1) * 128],
                        emb[:, kc, :], start=(kc == 0), stop=(kc == HC - 1),
                        skip_group_check=True)
                nc.vector.tensor_scalar(
                    out=pc_sb[:, mc, :], in0=ps[:],
                    scalar1=bc_sb[:, mc:mc + 1], scalar2=None, op0=ALU.add)

            # pa_sb[:, sc, t] = (emb_seq @ A1.T).T + attn_b
            for sc in range(SC):
                rows = 128 if sc < SC - 1 else s_last
                ps = pps.tile([128, T], F32, tag="pp")
                for kc in range(HC):
                    nc.tensor.matmul(
                        ps[0:rows, :], a1[:, kc, sc * 128:sc * 128 + rows],
                        emb[:, kc, :], start=(kc == 0), stop=(kc == HC - 1),
                        skip_group_check=True)
                nc.vector.tensor_scalar(
                    out=pa_sb[0:rows, sc, :], in0=ps[0:rows, :],
                    scalar1=ba_sb[0:rows, sc:sc + 1], scalar2=None,
                    op0=ALU.add)

        # ----------------- encoder recurrence -----------------
        def gru_gates(psr, psn, psz, rz_bias_r, rz_bias_z, hn_bias, xn_src,
                      h_prev, store_to, t):
            """Shared GRU gate math. psr/psn/psz are [128, HC] psum regions
            (already contain W@x accumulations); xn_src yields the n-gate
            x-part tensor; store_to is the bf16 [128, HC] destination slice."""
            ar = work.tile([128, HC], F32, tag="ar")
            nc.vector.tensor_tensor(out=ar[:], in0=psr, in1=rz_bias_r,
                                    op=ALU.add)
            rt = work.tile([128, HC], F32, tag="rt")
            nc.scalar.activation(rt[:], ar[:], AF.Tanh, scale=0.5)
            r = work.tile([128, HC], F32, tag="r")
            nc.vector.tensor_scalar(out=r[:], in0=rt[:], scalar1=0.5,
                                    scalar2=0.5, op0=ALU.mult, op1=ALU.add)
            u = work.tile([128, HC], F32, tag="u")
            nc.vector.tensor_tensor(out=u[:], in0=psn, in1=hn_bias, op=ALU.add)
            t1 = work.tile([128, HC], F32, tag="t1")
            nc.vector.tensor_tensor(out=t1[:], in0=r[:], in1=u[:], op=ALU.mult)
            t2 = work.tile([128, HC], F32, tag="t2")
            nc.vector.tensor_tensor(out=t2[:], in0=t1[:], in1=xn_src,
                                    op=ALU.add)
            n = work.tile([128, HC], F32, tag="n")
            nc.scalar.activation(n[:], t2[:], AF.Tanh)
            az = work.tile([128, HC], F32, tag="az")
            nc.vector.tensor_tensor(out=az[:], in0=psz, in1=rz_bias_z,
                                    op=ALU.add)
            zt = work.tile([128, HC], F32, tag="zt")
            nc.scalar.activation(zt[:], az[:], AF.Tanh, scale=0.5)
            z = work.tile([128, HC], F32, tag="z")
            nc.vector.tensor_scalar(out=z[:], in0=zt[:], scalar1=0.5,
                                    scalar2=0.5, op0=ALU.mult, op1=ALU.add)
            d = work.tile([128, HC], F32, tag="d")
            nc.vector.tensor_tensor(out=d[:], in0=h_prev[:], in1=n[:],
                                    op=ALU.subtract)
            zd = work.tile([128, HC], F32, tag="zd")
            nc.vector.tensor_tensor(out=zd[:], in0=z[:], in1=d[:],
                                    op=ALU.mult)
            h_new = hpool.tile([128, HC], F32, tag="h")
            nc.vector.tensor_tensor(out=h_new[:], in0=n[:], in1=zd[:],
                                    op=ALU.add)
            hbf = work.tile([128, HC], BF, tag="hbf")
            nc.vector.tensor_copy(hbf[:], h_new[:])
            nc.vector.tensor_copy(store_to, hbf[:])
            return h_new, hbf

        encp_cm = tc.tile_pool(name="encpsum", bufs=2, space="PSUM")
        encp = encp_cm.__enter__()

        # t = 0 (h = 0: all W@h terms vanish; gx already contains biases)
        h0 = hpool.tile([128, HC], F32, tag="h")
        rt0 = work.tile([128, HC], F32, tag="rt")
        nc.scalar.activation(rt0[:], gxp[:, 0:HC, 0], AF.Tanh, scale=0.5)
        r0 = work.tile([128, HC], F32, tag="r")
        nc.vector.tensor_scalar(out=r0[:], in0=rt0[:], scalar1=0.5,
                                scalar2=0.5, op0=ALU.mult, op1=ALU.add)
        t10 = work.tile([128, HC], F32, tag="t1")
        nc.vector.tensor_tensor(out=t10[:], in0=r0[:], in1=bhn_e[:],
                                op=ALU.mult)
        t20 = work.tile([128, HC], F32, tag="t2")
        nc.vector.tensor_tensor(out=t20[:], in0=t10[:], in1=gxp[:, 2 * HC:3 * HC, 0],
                                op=ALU.add)
        n0 = work.tile([128, HC], F32, tag="n")
        nc.scalar.activation(n0[:], t20[:], AF.Tanh)
        zt0 = work.tile([128, HC], F32, tag="zt")
        nc.scalar.activation(zt0[:], gxp[:, HC:2 * HC, 0], AF.Tanh, scale=0.5)
        z0 = work.tile([128, HC], F32, tag="z")
        nc.vector.tensor_scalar(out=z0[:], in0=zt0[:], scalar1=0.5,
                                scalar2=0.5, op0=ALU.mult, op1=ALU.add)
        d0 = work.tile([128, HC], F32, tag="d")
        nc.vector.tensor_scalar(out=d0[:], in0=n0[:], scalar1=-1.0,
                                scalar2=None, op0=ALU.mult)
        zd0 = work.tile([128, HC], F32, tag="zd")
        nc.vector.tensor_tensor(out=zd0[:], in0=z0[:], in1=d0[:], op=ALU.mult)
        nc.vector.tensor_tensor(out=h0[:], in0=n0[:], in1=zd0[:], op=ALU.add)
        hbf0 = work.tile([128, HC], BF, tag="hbf")
        nc.vector.tensor_copy(hbf0[:], h0[:])
        nc.vector.tensor_copy(enc_outsT[:, 0:HC, 0], hbf0[:])
        h_prev, hbf_prev = h0, hbf0

        for t in range(1, T):
            psr = encp.tile([128, HC], F32, tag="psr")
            psn = encp.tile([128, HC], F32, tag="psn")
            psz = encp.tile([128, HC], F32, tag="psz")
            for ps, glo in ((psr, 0), (psn, 2 * HC), (psz, HC)):
                for c in range(HC):
                    gc = glo + c
                    for kc in range(HC):
                        nc.tensor.matmul(
                            ps[:, c:c + 1],
                            whhe[:, kc, gc * 128:(gc + 1) * 128],
                            hbf_prev[:, kc:kc + 1],
                            start=(kc == 0), stop=(kc == HC - 1),
                            skip_group_check=True)
            h_prev, hbf_prev = gru_gates(
                psr[:], psn[:], psz[:],
                gxp[:, 0:HC, t], gxp[:, HC:2 * HC, t], bhn_e[:],
                gxp[:, 2 * HC:3 * HC, t],
                h_prev, enc_outsT[:, 0:HC, t], t)

        encp_cm.__exit__(None, None, None)

        # ----------------- M2 = enc_outs @ C2.T -----------------
        m2p_cm = tc.tile_pool(name="m2psum", bufs=4, space="PSUM")
        m2p = m2p_cm.__enter__()
        for tc_i in range(TC):
            rows = 128 if tc_i < TC - 1 else s_last
            for n0 in range(0, H, NV):
                ps = m2p.tile([128, NV], F32, tag="m2p")
                for kc in range(HC):
                    nc.tensor.matmul(
                        ps[0:rows, :],
                        enc_outsT[:, kc, tc_i * 128:tc_i * 128 + rows],
                        c2[:, kc, n0:n0 + NV],
                        start=(kc == 0), stop=(kc == HC - 1),
                        skip_group_check=True)
                nc.vector.tensor_copy(m2_sb[0:rows, tc_i, n0:n0 + NV],
                                      ps[0:rows, :])
        m2p_cm.__exit__(None, None, None)
        encw_cm.__exit__(None, None, None)

        # early half of out_W streams in under the decoder loop; it outlives
        # the dec-phase weights into the projection phase
        oww_cm = tc.tile_pool(name="oww", bufs=1)
        oww = oww_cm.__enter__()
        owa = oww.tile([128, HC, VA], BF, tag="owa")
        nc.sync.dma_start(owa[:], out_wta[:])

        # dec-phase stationary weights
        decw_cm = tc.tile_pool(name="decw", bufs=1)
        decw = decw_cm.__enter__()
        wihd = decw.tile([128, HC, G], BF, tag="wihd")
        wstk = decw.tile([128, HC, GS], BF, tag="wstk")
        nc.sync.dma_start(wihd[:], wihd_t[:])
        nc.sync.dma_start(wstk[:], wstk_t[:])

        # ----------------- decoder recurrence -----------------
        decp_cm = tc.tile_pool(name="decpsum", bufs=2, space="PSUM")
        decp = decp_cm.__enter__()
        COL_S = SC + 8           # S-sum column in psum_RA

        for t in range(T):
            # PSUM accumulation here spans temporally separated matmul groups
            # (Whh@h early, Wih@comb late). A start=True anywhere in a bank
            # clears the per-element has_written flags bank-wide, which would
            # turn the late group's accumulation into overwrite. So: memset
            # the tiles and use start=False on every decoder matmul —
            # correct regardless of flag state or scheduling order.
            psra = decp.tile([128, SC + 9], F32, tag="psra")
            psn = decp.tile([128, HC], F32, tag="psn")
            psz = decp.tile([128, HC], F32, tag="psz")
            psb = decp.tile([128, 9], F32, tag="psb")
            nc.vector.memset(psra[:], 0.0)
            nc.vector.memset(psn[:], 0.0)
            nc.vector.memset(psz[:], 0.0)
            nc.vector.memset(psb[:], 0.0)

            if t == 0:
                hsrc_bf = work.tile([128, HC], BF, tag="hbf2")
                nc.vector.tensor_copy(hsrc_bf[:], enc_outsT[:, 0:HC, T - 1])
            else:
                hsrc_bf = hbf_prev

            # --- stacked [dec_Whh; A2] @ h: attention rows first ---
            for st in range(GC, GSC):
                lo = st * 128
                rows = min(128, GS - lo)
                sc = st - GC
                for kc in range(HC):
                    nc.tensor.matmul(
                        psra[0:rows, sc:sc + 1],
                        wstk[:, kc, lo:lo + rows],
                        hsrc_bf[:, kc:kc + 1],
                        start=False, stop=(kc == HC - 1),
                        skip_group_check=True)
            # r rows (psum_RA cols SC..SC+8), n rows, z rows; psn is only
            # written by this pass so its group closes here, r/z groups stay
            # open for the dec_Wih contributions below.
            for ps, glo, coff, closes in ((psra, 0, SC, False),
                                          (psn, 2 * HC, 0, True),
                                          (psz, HC, 0, False)):
                for c in range(HC):
                    gc = glo + c
                    for kc in range(HC):
                        nc.tensor.matmul(
                            ps[:, coff + c:coff + c + 1],
                            wstk[:, kc, gc * 128:(gc + 1) * 128],
                            hsrc_bf[:, kc:kc + 1],
                            start=False,
                            stop=(closes and kc == HC - 1),
                            skip_group_check=True)

            # --- attention softmax (division-free) ---
            s_sb = work.tile([128, SC], F32, tag="s")
            e_bf = work.tile([128, SC], BF, tag="e")
            acc = work.tile([128, SC], F32, tag="acc")
            for sc in range(SC):
                rows = 128 if sc < SC - 1 else s_last
                nc.vector.tensor_tensor(
                    out=s_sb[0:rows, sc:sc + 1],
                    in0=psra[0:rows, sc:sc + 1],
                    in1=pa_sb[0:rows, sc, t:t + 1], op=ALU.add)
                nc.scalar.activation(
                    e_bf[0:rows, sc:sc + 1], s_sb[0:rows, sc:sc + 1],
                    AF.Exp, accum_out=acc[0:rows, sc:sc + 1])
            for sc in range(SC):
                rows = 128 if sc < SC - 1 else s_last
                nc.tensor.matmul(
                    psra[0:1, COL_S:COL_S + 1], ones_col[0:rows, :],
                    acc[0:rows, sc:sc + 1],
                    start=False, stop=(sc == SC - 1),
                    skip_group_check=True)
            # q = e @ M2  -> psum_B cols 0:8
            for mc in range(HC):
                for tc_i in range(TC):
                    rows = 128 if tc_i < TC - 1 else s_last
                    nc.tensor.matmul(
                        psb[:, mc:mc + 1],
                        m2_sb[0:rows, tc_i, mc * 128:(mc + 1) * 128],
                        e_bf[0:rows, tc_i:tc_i + 1],
                        start=False, stop=(tc_i == TC - 1),
                        skip_group_check=True)
            rs = work.tile([1, 1], F32, tag="rs")
            nc.vector.reciprocal(rs[:], psra[0:1, COL_S:COL_S + 1])
            rs_bf = work.tile([1, 1], BF, tag="rsbf")
            nc.vector.tensor_copy(rs_bf[:], rs[:])
            nc.tensor.matmul(psb[:, 8:9], ones_bf[:], rs_bf[:],
                             start=False, stop=True, skip_group_check=True)
            rs_col = work.tile([128, 1], F32, tag="rscol")
            nc.vector.tensor_copy(rs_col[:], psb[:, 8:9])
            q1 = work.tile([128, HC], F32, tag="q1")
            nc.vector.tensor_scalar(out=q1[:], in0=psb[:, 0:HC],
                                    scalar1=rs_col[:], scalar2=None,
                                    op0=ALU.mult)
            q2 = work.tile([128, HC], F32, tag="q2")
            nc.vector.tensor_tensor(out=q2[:], in0=q1[:],
                                    in1=pc_sb[:, 0:HC, t], op=ALU.add)
            comb_bf = work.tile([128, HC], BF, tag="comb")
            nc.scalar.activation(comb_bf[:], q2[:], AF.Relu)

            # psb cols 0:8 are reused for gx_n after q was read — clear again
            nc.vector.memset(psb[:, 0:HC], 0.0)

            # --- dec_Wih @ comb: finish r, then n (psum_B), then z ---
            for c in range(HC):
                for kc in range(HC):
                    nc.tensor.matmul(
                        psra[:, SC + c:SC + c + 1],
                        wihd[:, kc, c * 128:(c + 1) * 128],
                        comb_bf[:, kc:kc + 1],
                        start=False, stop=(kc == HC - 1),
                        skip_group_check=True)
            for c in range(HC):
                gc = 2 * HC + c
                for kc in range(HC):
                    nc.tensor.matmul(
                        psb[:, c:c + 1],
                        wihd[:, kc, gc * 128:(gc + 1) * 128],
                        comb_bf[:, kc:kc + 1],
                        start=False, stop=(kc == HC - 1),
                        skip_group_check=True)
            for c in range(HC):
                gc = HC + c
                for kc in range(HC):
                    nc.tensor.matmul(
                        psz[:, c:c + 1],
                        wihd[:, kc, gc * 128:(gc + 1) * 128],
                        comb_bf[:, kc:kc + 1],
                        start=False, stop=(kc == HC - 1),
                        skip_group_check=True)

            xn = work.tile([128, HC], F32, tag="xn")
            nc.vector.tensor_tensor(out=xn[:], in0=psb[:, 0:HC],
                                    in1=bxn_d[:], op=ALU.add)
            h_prev, hbf_prev = gru_gates(
                psra[:, SC:SC + 8], psn[:], psz[:],
                brz_d[:, 0:HC], brz_d[:, HC:2 * HC], bhn_d[:], xn[:],
                h_prev, h_decT[:, 0:HC, t], t)

        decp_cm.__exit__(None, None, None)
        decw_cm.__exit__(None, None, None)

        # ----------------- projection + log_softmax -----------------
        projw_cm = tc.tile_pool(name="projw", bufs=1)
        projw = projw_cm.__enter__()
        owb = projw.tile([128, HC, VS - VA], BF, tag="owb")
        nc.sync.dma_start(owb[:], out_wtb[:])
        outb_sb = projw.tile([1, VS], BF, tag="outb")
        nc.sync.dma_start(outb_sb[:], out_bb[:])
        sacc = projw.tile([128, TC, VC], F32, tag="sacc")
        s_loc = projw.tile([128, TC], F32, tag="sloc")
        logz = projw.tile([128, TC], F32, tag="logz")
        nc.vector.memset(sacc[:], 0.0)

        def w_slice(j):
            v0 = j * NV
            nv = min(NV, VS - v0)
            if v0 < VA:
                return owa, v0, nv
            return owb, v0 - VA, nv

        pj1_cm = tc.tile_pool(name="pj1", bufs=4, space="PSUM")
        pj1 = pj1_cm.__enter__()
        scr_cm = tc.tile_pool(name="scr", bufs=3)
        scr = scr_cm.__enter__()

        def proj_mms(ps, m, rows, j, tagsfx):
            src, off, nv = w_slice(j)
            v0 = j * NV
            nc.tensor.matmul(ps[0:rows, 0:nv], ones_bf[0:1, 0:rows],
                             outb_sb[0:1, v0:v0 + nv],
                             start=True, stop=False, skip_group_check=True)
            for kc in range(HC):
                nc.tensor.matmul(
                    ps[0:rows, 0:nv],
                    h_decT[:, kc, m * 128:m * 128 + rows],
                    src[:, kc, off:off + nv],
                    start=False, stop=(kc == HC - 1), skip_group_check=True)
            return nv

        for m in range(TC):
            rows = 128 if m < TC - 1 else s_last
            for j in range(VC):
                ps = pj1.tile([128, NV], F32, tag="pj1")
                nv = proj_mms(ps, m, rows, j, "a")
                eb = scr.tile([128, NV], BF, tag="escr")
                nc.scalar.activation(eb[0:rows, 0:nv], ps[0:rows, 0:nv],
                                     AF.Exp, accum_out=sacc[0:rows, m, j:j + 1])
        nc.vector.reduce_sum(s_loc[:], sacc[:], axis=mybir.AxisListType.X)

        with tc.tile_pool(name="dram", bufs=1, space="DRAM") as dram:
            ib = dram.tile([128, TC], F32)
            ob = dram.tile([128, TC], F32)
            nc.gpsimd.dma_start(ib[:], s_loc[:])
            nc.gpsimd.collective_compute(
                "AllReduce", ALU.add,
                replica_groups=[list(range(N_CORES))],
                ins=[ib.opt()], outs=[ob.opt()])
            s_tot = projw.tile([128, TC], F32, tag="stot")
            nc.sync.dma_start(s_tot[:], ob[:])
            nc.scalar.activation(logz[:], s_tot[:], AF.Ln)

            for m in range(TC):
                rows = 128 if m < TC - 1 else s_last
                for j in range(VC):
                    ps = pj1.tile([128, NV], F32, tag="pj2")
                    nv = proj_mms(ps, m, rows, j, "b")
                    ot = scr.tile([128, NV], F32, tag="oscr")
                    nc.vector.tensor_scalar(
                        out=ot[0:rows, 0:nv], in0=ps[0:rows, 0:nv],
                        scalar1=logz[0:rows, m:m + 1], scalar2=None,
                        op0=ALU.subtract)
                    v0 = j * NV
                    nc.sync.dma_start(
                        out_d[m * 128:m * 128 + rows, v0:v0 + nv],
                        ot[0:rows, 0:nv])

        nc.sync.dma_start(dbg_enc[:],
                          enc_outsT[:].rearrange("p a b -> p (a b)"))
        nc.sync.dma_start(dbg_hdec[:],
                          h_decT[:].rearrange("p a b -> p (a b)"))

        scr_cm.__exit__(None, None, None)
        pj1_cm.__exit__(None, None, None)
        projw_cm.__exit__(None, None, None)
        oww_cm.__exit__(None, None, None)
        work_cm.__exit__(None, None, None)
        hpool_cm.__exit__(None, None, None)
        cons_cm.__exit__(None, None, None)

    nc.compile()
    return nc


# ---------------------------------------------------------------------------
# host side
# ---------------------------------------------------------------------------

def _tiles(M):
    """M [rows, H] -> lhsT tile layout [128, HC*rows] bf16 (M.T tiled)."""
    rows = M.shape[0]
    return np.ascontiguousarray(
        M.T.reshape(HC, 128, rows).transpose(1, 0, 2).reshape(128, HC * rows)
    ).astype(BF16)


def _cols(v):
    """v [C*128] -> [128, C] fp32 column layout."""
    C = v.shape[0] // 128
    return np.ascontiguousarray(v.reshape(C, 128).T).astype(np.float32)


_PROG_CACHE = {}


def _get_program(T, VS, VA):
    key = (T, VS, VA)
    if key not in _PROG_CACHE:
        _PROG_CACHE[key] = build_program(T, VS, VA)
    return _PROG_CACHE[key]


def prepare_inputs(inputs, T, VS, VA):
    f32 = np.float32
    inp = np.asarray(inputs["input_seq"], f32)[:, 0, :]      # [T, H]
    target = np.asarray(inputs["target"]).astype(np.int64)[:, 0]
    emb_dec = np.asarray(inputs["emb_dec"], f32)
    toks = np.concatenate([[SOS], target[:-1]])
    emb_seq = emb_dec[toks]                                   # [T, H]

    attn_W = np.asarray(inputs["attn_W"], f32)
    A1, A2 = attn_W[:, :H], attn_W[:, H:]
    comb_W = np.asarray(inputs["comb_W"], f32)
    C1, C2 = comb_W[:, :H], comb_W[:, H:]
    enc_bih = np.asarray(inputs["enc_bih"], f32)
    enc_bhh = np.asarray(inputs["enc_bhh"], f32)
    dec_bih = np.asarray(inputs["dec_bih"], f32)
    dec_bhh = np.asarray(inputs["dec_bhh"], f32)
    attn_b = np.asarray(inputs["attn_b"], f32)
    comb_b = np.asarray(inputs["comb_b"], f32)
    out_W = np.asarray(inputs["out_W"], f32)
    out_b = np.asarray(inputs["out_b"], f32)

    SC = _cdiv(T, 128)
    stk = np.concatenate([np.asarray(inputs["dec_Whh"], f32), A2], axis=0)

    ve = np.concatenate([(enc_bih + enc_bhh)[:2 * H], enc_bih[2 * H:]])
    attn_b_pad = np.zeros(SC * 128, f32)
    attn_b_pad[:T] = attn_b

    shared = {
        "wihe_t": _tiles(np.asarray(inputs["enc_Wih"], f32)),
        "whhe_t": _tiles(np.asarray(inputs["enc_Whh"], f32)),
        "wihd_t": _tiles(np.asarray(inputs["dec_Wih"], f32)),
        "wstk_t": _tiles(stk),
        "c1t": _tiles(C1),
        "c2t": _tiles(C2),
        "a1t": _tiles(A1),
        "embt": _tiles(emb_seq),
        "inpt": _tiles(inp),
        "bias_e": _cols(ve),
        "bhh_n_e": _cols(enc_bhh[2 * H:]),
        "biasd_rz": _cols((dec_bih + dec_bhh)[:2 * H]),
        "biasd_hn": _cols(dec_bhh[2 * H:]),
        "biasd_xn": _cols(dec_bih[2 * H:]),
        "bias_a": _cols(attn_b_pad),
        "bias_c": _cols(comb_b),
    }

    V = out_W.shape[0]
    in_maps = []
    for c in range(N_CORES):
        vlo = c * VS
        vhi = min(V, vlo + VS)
        Wsh = np.zeros((VS, H), f32)
        bsh = np.full(VS, -1e30, f32)
        if vhi > vlo:
            Wsh[:vhi - vlo] = out_W[vlo:vhi]
            bsh[:vhi - vlo] = out_b[vlo:vhi]
        wt = _tiles(Wsh)                    # [128, HC*VS]
        wt3 = wt.reshape(128, HC, VS)
        m = dict(shared)
        m["out_wta"] = np.ascontiguousarray(wt3[:, :, :VA]).reshape(128, HC * VA)
        m["out_wtb"] = np.ascontiguousarray(wt3[:, :, VA:]).reshape(
            128, HC * (VS - VA))
        m["out_bb"] = bsh.astype(BF16)[None, :]
        in_maps.append(m)
    return in_maps


def run(inputs, T=256, VS=None, VA=None, trace=False):
    V = np.asarray(inputs["out_W"]).shape[0]
    if VS is None:
        VS = _cdiv(_cdiv(V, N_CORES), NV) * NV   # 6656 for V=50257
    if VA is None:
        VA = max(NV, (VS // (2 * NV)) * NV)
    nc = _get_program(T, VS, VA)
    in_maps = prepare_inputs(inputs, T, VS, VA)
    res = run_bass_kernel_spmd(nc, in_maps, core_ids=list(range(N_CORES)),
                               trace=trace)
    V = np.asarray(inputs["out_W"]).shape[0]
    parts = []
    for c in range(N_CORES):
        vlo = c * VS
        vhi = min(V, vlo + VS)
        if vhi <= vlo:
            continue
        parts.append(res.results[c]["out"][:, :vhi - vlo])
    full = np.concatenate(parts, axis=1).astype(np.float32)
    return full.reshape(T, 1, V), res


def make_runner(nc):
    """Cached PJRT dispatcher mirroring bass2jax.run_bass_via_pjrt, but the
    jitted executable is built once and reused across calls (for timing)."""
    import jax
    from jax.experimental.shard_map import shard_map
    from jax.sharding import Mesh, PartitionSpec
    from concourse import bass2jax
    from concourse import mybir as _mybir

    bass2jax.install_neuronx_cc_hook()
    partition_name = (nc.partition_id_tensor.name
                      if nc.partition_id_tensor else None)
    in_names, out_names, out_avals, zero_outs = [], [], [], []
    for alloc in nc.m.functions[0].allocations:
        if not isinstance(alloc, _mybir.MemoryLocationSet):
            continue
        name = alloc.memorylocations[0].name
        if alloc.kind == "ExternalInput":
            if name != partition_name:
                in_names.append(name)
        elif alloc.kind == "ExternalOutput":
            out_names.append(name)
            shape = tuple(alloc.tensor_shape)
            dtype = _mybir.dt.np(alloc.dtype)
            out_avals.append(jax.core.ShapedArray(shape, dtype))
            zero_outs.append(np.zeros(shape, dtype))
    n_params = len(in_names)
    n_outs = len(out_avals)
    all_names = in_names + out_names
    if partition_name is not None:
        all_names = all_names + [partition_name]
    donate = tuple(range(n_params, n_params + n_outs))

    def _body(*args):
        operands = list(args)
        if partition_name is not None:
            operands.append(bass2jax.partition_id_tensor())
        outs = bass2jax._bass_exec_p.bind(
            *operands,
            out_avals=tuple(out_avals),
            in_names=tuple(all_names),
            out_names=tuple(out_names),
            lowering_input_output_aliases=(),
            sim_require_finite=True,
            sim_require_nnan=True,
            nc=nc,
        )
        return tuple(outs)

    devices = jax.devices()[:N_CORES]
    mesh = Mesh(np.asarray(devices), ("core",))
    in_specs = (PartitionSpec("core"),) * (n_params + n_outs)
    out_specs = (PartitionSpec("core"),) * n_outs
    sharding = jax.sharding.NamedSharding(mesh, PartitionSpec("core"))
    sharded = jax.jit(
        shard_map(_body, mesh=mesh, in_specs=in_specs, out_specs=out_specs,
                  check_rep=False),
        keep_unused=True)

    cache = {}

    def call(in_maps, fetch=True):
        key = id(in_maps)
        if key not in cache:
            concat_in = [
                jax.device_put(
                    np.concatenate([np.asarray(in_maps[c][nm]) for c in
                                    range(N_CORES)], axis=0), sharding)
                for nm in in_names
            ]
            concat_zeros = [
                jax.device_put(
                    np.zeros((N_CORES * z.shape[0], *z.shape[1:]), z.dtype),
                    sharding)
                for z in zero_outs
            ]
            jax.block_until_ready(concat_in)
            jax.block_until_ready(concat_zeros)
            cache.clear()
            cache[key] = (concat_in, concat_zeros)
        concat_in, concat_zeros = cache[key]
        out_arrs = sharded(*concat_in, *concat_zeros)
        jax.block_until_ready(out_arrs)
        if not fetch:
            return None
        out_arrs = [np.asarray(o) for o in out_arrs]
        return [
            {nm: out_arrs[i].reshape(N_CORES, *out_avals[i].shape)[c]
             for i, nm in enumerate(out_names)}
            for c in range(N_CORES)
        ]

    return call


def kernel(**inputs):
    out, _ = run(inputs, T=256)
    return out



# revision 15
# speedup vs baseline: 3988.6559x; 3988.6559x over previous
"""Trainium2 Bass kernel for nn_AttnAutoEncoderRNN (H=1024, V=50257, T=256).

Strategy:
  - The GRU encoder/decoder recurrence is inherently sequential (batch=1), so
    it is replicated on all 8 cores (per-step cross-core sync is impossible:
    the on-device AllReduce floor ~10us exceeds a whole step).
  - The big [V,H] output projection + log_softmax is vocab-sharded across the
    8 cores; a single tiny AllReduce combines the per-shard exp-sums.
  - Recurrence weights (enc_Whh, dec_Whh+A2, dec_Wih, M2) are fp8-e4m3 with
    an exact power-of-2 scale S=256 folded into host-scaled biases and the
    activation `scale=` immediates. The ~600 N=1 matvec matmuls per step are
    issue-bound (~40ns/LDW+MM pair, measured dtype-independent), so fp8's
    win is halved weight SBUF/DMA/transfer — which is what lets both
    projection halves plus the fp16 logit cache stay resident. Everything
    else stays bf16 with fp32 PSUM accumulation.
  - sigmoid(x) = 0.5*tanh(x/2)+0.5 keeps the whole recurrence in one ACT
    table set; r and z share one fused [128,16] tanh.
  - Attention context folded: M2 = enc_outs @ C2.T is precomputed once (and
    cast to fp8 on device), so a decoder step needs e @ M2 instead of ctx
    and C2 @ ctx. Division-free softmax via a broadcast reciprocal.
  - Projection runs ONE matmul pass; logits are cached in fp16 in SBUF, and
    after the exp-sum AllReduce only a subtract+DMA pass remains.
"""

import numpy as np
import ml_dtypes

import concourse.bass as bass
import concourse.bacc as bacc
import concourse.tile as tile
import concourse.mybir as mybir
from concourse.bass_utils import run_bass_kernel_spmd

BF16 = ml_dtypes.bfloat16
FP8 = ml_dtypes.float8_e4m3
F32 = mybir.dt.float32
F16 = mybir.dt.float16
BF = mybir.dt.bfloat16
E4 = mybir.dt.float8e4
AF = mybir.ActivationFunctionType
ALU = mybir.AluOpType

H = 1024
HC = H // 128            # 8 k-chunks of the hidden dim
G = 3 * H                # 3072 gate rows
GC = G // 128            # 24 gate m-tiles
V_FULL = 50257
N_CORES = 8
SOS = 1
NV = 512                 # vocab tile width in the projection
S = 256.0                # fp8 weight scale (power of 2: exact on bf16/fp32)
SM = 32.0                # fp8 scale for the M2 attention matrix


def _cdiv(a, b):
    return (a + b - 1) // b


def build_program(T, VS, VA, reps=1, n_cores=N_CORES, debug_outs=False):
    """T timesteps, VS = padded vocab shard, VA = part of VS loaded early.
    reps>1 re-emits the whole body for marginal device-time measurement."""
    SC = _cdiv(T, 128)             # chunks of the attention (T) axis
    TC = SC                        # time chunks (projection M-tiles)
    s_last = T - (SC - 1) * 128
    VC = _cdiv(VS, NV)
    GS = G + T                     # stacked [dec_Whh; A2] rows
    GSC = _cdiv(GS, 128)
    RZC = 2 * HC                   # fused r+z columns

    nc = bacc.Bacc("TRN2", target_bir_lowering=False, debug=False,
                   num_devices=n_cores)

    def din(name, shape, dt):
        return nc.dram_tensor(name, shape, dt, kind="ExternalInput").ap()

    whhe_t = din("whhe_t", [128, HC * G], E4)
    wihd_t = din("wihd_t", [128, HC * G], E4)
    wstk_t = din("wstk_t", [128, HC * GS], E4)
    wihe_t = din("wihe_t", [128, HC * G], BF)
    c1t = din("c1t", [128, HC * H], BF)
    c2t = din("c2t", [128, HC * H], BF)
    a1t = din("a1t", [128, HC * T], BF)
    embt = din("embt", [128, HC * T], BF)
    inpt = din("inpt", [128, HC * T], BF)
    bias_e = din("bias_e", [128, GC], F32)
    bhh_n_e = din("bhh_n_e", [128, HC], F32)
    biasd_rz = din("biasd_rz", [128, RZC], F32)
    biasd_hnxn = din("biasd_hnxn", [128, RZC], F32)
    bias_a = din("bias_a", [128, SC], F32)
    bias_c = din("bias_c", [128, HC], F32)
    out_wta = din("out_wta", [128, HC * VA], BF)
    out_wtb = din("out_wtb", [128, HC * (VS - VA)], BF)
    out_bb = din("out_bb", [1, VS], BF)

    out_d = nc.dram_tensor("out", [T, VS], F32, kind="ExternalOutput").ap()
    if debug_outs:
        dbg_enc = nc.dram_tensor("dbg_enc", [128, HC * T], BF,
                                 kind="ExternalOutput").ap()
        dbg_hdec = nc.dram_tensor("dbg_hdec", [128, HC * T], BF,
                                  kind="ExternalOutput").ap()

    with tile.TileContext(nc) as tc:
        # ----------------- persistent tiles -----------------
        cons_cm = tc.tile_pool(name="cons", bufs=1)
        cons = cons_cm.__enter__()
        enc_outsT = cons.tile([128, HC, T], BF, tag="enc_outsT")
        h_decT = cons.tile([128, HC, T], BF, tag="h_decT")
        m2_sb = cons.tile([128, TC, H], E4, tag="m2")
        pc_sb = cons.tile([128, HC, T], F32, tag="pc")
        pa_sb = cons.tile([128, SC, T], F32, tag="pa")
        be_sb = cons.tile([128, GC], F32, tag="be")
        bhn_e = cons.tile([128, HC], F32, tag="bhne")
        brz_d = cons.tile([128, RZC], F32, tag="brzd")
        bhnxn_d = cons.tile([128, RZC], F32, tag="bhnxnd")
        ba_sb = cons.tile([128, SC], F32, tag="ba")
        bc_sb = cons.tile([128, HC], F32, tag="bc")
        ones_col = cons.tile([128, 1], F32, tag="ones_col")
        ones_bf = cons.tile([1, 128], BF, tag="ones_bf")
        sm_row = cons.tile([1, 128], BF, tag="sm_row")

        nc.sync.dma_start(be_sb[:], bias_e[:])
        nc.sync.dma_start(bhn_e[:], bhh_n_e[:])
        nc.sync.dma_start(brz_d[:], biasd_rz[:])
        nc.sync.dma_start(bhnxn_d[:], biasd_hnxn[:])
        nc.sync.dma_start(ba_sb[:], bias_a[:])
        nc.sync.dma_start(bc_sb[:], bias_c[:])
        nc.vector.memset(ones_col[:], 1.0)
        nc.vector.memset(ones_bf[:], 1.0)
        nc.vector.memset(sm_row[:], 1.0 / SM)

        # small per-step work tiles
        work_cm = tc.tile_pool(name="work", bufs=3)
        work = work_cm.__enter__()

        for rep in range(reps):
            # enc-phase tensors (freed after M2)
            encw_cm = tc.tile_pool(name="encw", bufs=1)
            encw = encw_cm.__enter__()
            whhe = encw.tile([128, HC, G], E4, tag="whhe")
            gxp = encw.tile([128, GC, T], F32, tag="gxp")
            c2 = encw.tile([128, HC, H], BF, tag="c2")
            nc.sync.dma_start(whhe[:], whhe_t[:])
            nc.sync.dma_start(c2[:], c2t[:])

            # ----------------- precompute phase -----------------
            with tc.tile_pool(name="pre", bufs=1) as pre, \
                 tc.tile_pool(name="prepsum", bufs=6, space="PSUM") as pps:
                wihe = pre.tile([128, HC, G], BF, tag="wihe")
                c1 = pre.tile([128, HC, H], BF, tag="c1")
                a1 = pre.tile([128, HC, T], BF, tag="a1")
                emb = pre.tile([128, HC, T], BF, tag="emb")
                inp = pre.tile([128, HC, T], BF, tag="inp")
                nc.sync.dma_start(wihe[:], wihe_t[:])
                nc.sync.dma_start(c1[:], c1t[:])
                nc.sync.dma_start(a1[:], a1t[:])
                nc.sync.dma_start(emb[:], embt[:])
                nc.sync.dma_start(inp[:], inpt[:])

                # gxp[:, gc, t] = S*((input_seq @ enc_Wih.T).T + biases)
                # (wihe, bias_e are S-scaled on host)
                for gc in range(GC):
                    ps = pps.tile([128, T], F32, tag="pp")
                    for kc in range(HC):
                        nc.tensor.matmul(
                            ps[:], wihe[:, kc, gc * 128:(gc + 1) * 128],
                            inp[:, kc, :], start=(kc == 0), stop=(kc == HC - 1),
                            skip_group_check=True)
                    nc.vector.tensor_scalar(
                        out=gxp[:, gc, :], in0=ps[:],
                        scalar1=be_sb[:, gc:gc + 1], scalar2=None, op0=ALU.add)

                # pc_sb[:, mc, t] = (emb_seq @ C1.T).T + comb_b   (unscaled)
                for mc in range(HC):
                    ps = pps.tile([128, T], F32, tag="pp")
                    for kc in range(HC):
                        nc.tensor.matmul(
                            ps[:], c1[:, kc, mc * 128:(mc + 1) * 128],
                            emb[:, kc, :], start=(kc == 0), stop=(kc == HC - 1),
                            skip_group_check=True)
                    nc.vector.tensor_scalar(
                        out=pc_sb[:, mc, :], in0=ps[:],
                        scalar1=bc_sb[:, mc:mc + 1], scalar2=None, op0=ALU.add)

                # pa_sb[:, sc, t] = S*((emb_seq @ A1.T).T + attn_b)
                for sc in range(SC):
                    rows = 128 if sc < SC - 1 else s_last
                    ps = pps.tile([128, T], F32, tag="pp")
                    for kc in range(HC):
                        nc.tensor.matmul(
                            ps[0:rows, :], a1[:, kc, sc * 128:sc * 128 + rows],
                            emb[:, kc, :], start=(kc == 0), stop=(kc == HC - 1),
                            skip_group_check=True)
                    nc.vector.tensor_scalar(
                        out=pa_sb[0:rows, sc, :], in0=ps[0:rows, :],
                        scalar1=ba_sb[0:rows, sc:sc + 1], scalar2=None,
                        op0=ALU.add)

            # ----------------- encoder recurrence -----------------
            encp_cm = tc.tile_pool(name="encpsum", bufs=2, space="PSUM")
            encp = encp_cm.__enter__()

            # t = 0 (h = 0: W@h terms vanish; gxp is S-scaled with biases)
            # h is carried in bf16 (the matmul rhs format) — the final gate
            # op casts straight into enc_outsT / h_decT, no extra copy.
            rzt0 = work.tile([128, RZC], F32, tag="rzt")
            nc.scalar.activation(rzt0[:], gxp[:, 0:RZC, 0], AF.Tanh,
                                 scale=0.5 / S)
            rz0 = work.tile([128, RZC], F32, tag="rz")
            nc.vector.tensor_scalar(out=rz0[:], in0=rzt0[:], scalar1=0.5,
                                    scalar2=0.5, op0=ALU.mult, op1=ALU.add)
            t10 = work.tile([128, HC], F32, tag="t1")
            nc.vector.tensor_tensor(out=t10[:], in0=rz0[:, 0:HC], in1=bhn_e[:],
                                    op=ALU.mult)
            t20 = work.tile([128, HC], F32, tag="t2")
            nc.vector.tensor_tensor(out=t20[:], in0=t10[:],
                                    in1=gxp[:, RZC:GC, 0], op=ALU.add)
            n0 = work.tile([128, HC], F32, tag="n")
            nc.scalar.activation(n0[:], t20[:], AF.Tanh, scale=1.0 / S)
            d0 = work.tile([128, HC], F32, tag="d")
            nc.vector.tensor_scalar(out=d0[:], in0=n0[:], scalar1=-1.0,
                                    scalar2=None, op0=ALU.mult)
            zd0 = work.tile([128, HC], F32, tag="zd")
            nc.vector.tensor_tensor(out=zd0[:], in0=rz0[:, HC:RZC], in1=d0[:],
                                    op=ALU.mult)
            nc.vector.tensor_tensor(out=enc_outsT[:, 0:HC, 0], in0=n0[:],
                                    in1=zd0[:], op=ALU.add)

            def gru_tail(ps_rz, ps_n_u, gx_rz, gx_n, hsrc, store_to):
                """Shared gate math: rz psum + rz x-part, n-gate h-part (u
                pre-bias), n-gate x-part, bf16 h source, bf16 dest slice."""
                arz = work.tile([128, RZC], F32, tag="arz")
                nc.vector.tensor_tensor(out=arz[:], in0=ps_rz, in1=gx_rz,
                                        op=ALU.add)
                rzt = work.tile([128, RZC], F32, tag="rzt")
                nc.scalar.activation(rzt[:], arz[:], AF.Tanh, scale=0.5 / S)
                rz = work.tile([128, RZC], F32, tag="rz")
                nc.vector.tensor_scalar(out=rz[:], in0=rzt[:], scalar1=0.5,
                                        scalar2=0.5, op0=ALU.mult, op1=ALU.add)
                t1 = work.tile([128, HC], F32, tag="t1")
                nc.vector.tensor_tensor(out=t1[:], in0=rz[:, 0:HC], in1=ps_n_u,
                                        op=ALU.mult)
                t2 = work.tile([128, HC], F32, tag="t2")
                nc.vector.tensor_tensor(out=t2[:], in0=t1[:], in1=gx_n,
                                        op=ALU.add)
                n = work.tile([128, HC], F32, tag="n")
                nc.scalar.activation(n[:], t2[:], AF.Tanh, scale=1.0 / S)
                d = work.tile([128, HC], F32, tag="d")
                nc.vector.tensor_tensor(out=d[:], in0=hsrc, in1=n[:],
                                        op=ALU.subtract)
                zd = work.tile([128, HC], F32, tag="zd")
                nc.vector.tensor_tensor(out=zd[:], in0=rz[:, HC:RZC],
                                        in1=d[:], op=ALU.mult)
                nc.vector.tensor_tensor(out=store_to, in0=n[:], in1=zd[:],
                                        op=ALU.add)

            for t in range(1, T):
                hsrc = enc_outsT[:, 0:HC, t - 1]
                ps = encp.tile([128, GC], F32, tag="pse")
                for c in range(GC):
                    for kc in range(HC):
                        nc.tensor.matmul(
                            ps[:, c:c + 1],
                            whhe[:, kc, c * 128:(c + 1) * 128],
                            hsrc[:, kc:kc + 1],
                            start=(kc == 0), stop=(kc == HC - 1),
                            skip_group_check=True)
                u = work.tile([128, HC], F32, tag="u")
                nc.vector.tensor_tensor(out=u[:], in0=ps[:, RZC:GC],
                                        in1=bhn_e[:], op=ALU.add)
                gru_tail(ps[:, 0:RZC], u[:], gxp[:, 0:RZC, t],
                         gxp[:, RZC:GC, t], hsrc, enc_outsT[:, 0:HC, t])

            encp_cm.__exit__(None, None, None)

            # ----------------- M2 = enc_outs @ C2.T  (SM-scaled fp8) -------
            m2p_cm = tc.tile_pool(name="m2psum", bufs=4, space="PSUM")
            m2p = m2p_cm.__enter__()
            for tc_i in range(TC):
                rows = 128 if tc_i < TC - 1 else s_last
                for n0_ in range(0, H, NV):
                    ps = m2p.tile([128, NV], F32, tag="m2p")
                    for kc in range(HC):
                        nc.tensor.matmul(
                            ps[0:rows, :],
                            enc_outsT[:, kc, tc_i * 128:tc_i * 128 + rows],
                            c2[:, kc, n0_:n0_ + NV],
                            start=(kc == 0), stop=(kc == HC - 1),
                            skip_group_check=True)
                    nc.vector.tensor_copy(m2_sb[0:rows, tc_i, n0_:n0_ + NV],
                                          ps[0:rows, :])
            m2p_cm.__exit__(None, None, None)
            encw_cm.__exit__(None, None, None)

            # dec-phase stationary weights + both projection halves (their
            # DMAs hide under the decoder)
            oww_cm = tc.tile_pool(name="oww", bufs=1)
            oww = oww_cm.__enter__()
            owa = oww.tile([128, HC, VA], BF, tag="owa")
            nc.sync.dma_start(owa[:], out_wta[:])

            # owb + bias pool sits below decw on the pool stack (it outlives
            # the decoder weights); its DMA is issued after wstk/wihd so the
            # decoder's first steps aren't delayed.
            owbp_cm = tc.tile_pool(name="owbp", bufs=1)
            owbp = owbp_cm.__enter__()
            owb = owbp.tile([128, HC, VS - VA], BF, tag="owb")
            outb_sb = owbp.tile([1, VS], BF, tag="outb")

            decw_cm = tc.tile_pool(name="decw", bufs=1)
            decw = decw_cm.__enter__()
            wihd = decw.tile([128, HC, G], E4, tag="wihd")
            wstk = decw.tile([128, HC, GS], E4, tag="wstk")
            nc.sync.dma_start(wihd[:], wihd_t[:])
            nc.sync.dma_start(wstk[:], wstk_t[:])
            nc.sync.dma_start(owb[:], out_wtb[:])
            nc.sync.dma_start(outb_sb[:], out_bb[:])

            # ----------------- decoder recurrence -----------------
            # psA: [att(SC) | ssum(1) | rz(16)] — rz spans two temporally
            # separated matmul groups (Whh@h early, Wih@comb late); a
            # start=True anywhere in this bank would clear has_written
            # bank-wide mid-accumulation, so: memset + start=False on every
            # psA matmul (accumulate-onto-zero is correct for any flag state).
            # psB: [n(8) | q->gxn(8) | rs(1)] — all groups self-contained, so
            # plain start=True groups, no memset; cols 8:16 are reused for
            # gx_n after q is consumed.
            decpA_cm = tc.tile_pool(name="decpsA", bufs=2, space="PSUM")
            decpA = decpA_cm.__enter__()
            decpB_cm = tc.tile_pool(name="decpsB", bufs=2, space="PSUM")
            decpB = decpB_cm.__enter__()
            COL_S = SC
            RZ0 = SC + 1

            for t in range(T):
                psA = decpA.tile([128, RZ0 + RZC], F32, tag="psA")
                psB = decpB.tile([128, 2 * HC + 1], F32, tag="psB")
                nc.vector.memset(psA[:], 0.0)

                if t == 0:
                    hsrc = enc_outsT[:, 0:HC, T - 1]
                else:
                    hsrc = h_decT[:, 0:HC, t - 1]

                # --- stacked A2 attention rows @ h ---
                for st in range(GC, GSC):
                    lo = st * 128
                    rows = min(128, GS - lo)
                    sc = st - GC
                    for kc in range(HC):
                        nc.tensor.matmul(
                            psA[0:rows, sc:sc + 1],
                            wstk[:, kc, lo:lo + rows],
                            hsrc[:, kc:kc + 1],
                            start=False, stop=(kc == HC - 1),
                            skip_group_check=True)
                # --- rz rows of Whh @ h (chunks 0:16 of wstk) ---
                for c in range(RZC):
                    for kc in range(HC):
                        nc.tensor.matmul(
                            psA[:, RZ0 + c:RZ0 + c + 1],
                            wstk[:, kc, c * 128:(c + 1) * 128],
                            hsrc[:, kc:kc + 1],
                            start=False, stop=False,
                            skip_group_check=True)
                # --- n rows of Whh @ h -> psB[:, 0:8] ---
                for c in range(HC):
                    gc = RZC + c
                    for kc in range(HC):
                        nc.tensor.matmul(
                            psB[:, c:c + 1],
                            wstk[:, kc, gc * 128:(gc + 1) * 128],
                            hsrc[:, kc:kc + 1],
                            start=(kc == 0), stop=(kc == HC - 1),
                            skip_group_check=True)

                # --- attention softmax (division-free) ---
                s_sb = work.tile([128, SC], F32, tag="s")
                e_bf = work.tile([128, SC], BF, tag="e")
                if s_last == 128:
                    # T % 128 == 0: one fused [128, SC] pass, scalar accum
                    acc = work.tile([128, 1], F32, tag="acc")
                    nc.vector.tensor_tensor(
                        out=s_sb[:], in0=psA[:, 0:SC],
                        in1=pa_sb[:, 0:SC, t], op=ALU.add)
                    nc.scalar.activation(
                        e_bf[:], s_sb[:], AF.Exp, scale=1.0 / S,
                        accum_out=acc[:])
                    nc.tensor.matmul(
                        psA[0:1, COL_S:COL_S + 1], ones_col[:], acc[:],
                        start=False, stop=True, skip_group_check=True)
                else:
                    acc = work.tile([128, SC], F32, tag="acc")
                    for sc in range(SC):
                        rows = 128 if sc < SC - 1 else s_last
                        nc.vector.tensor_tensor(
                            out=s_sb[0:rows, sc:sc + 1],
                            in0=psA[0:rows, sc:sc + 1],
                            in1=pa_sb[0:rows, sc, t:t + 1], op=ALU.add)
                        nc.scalar.activation(
                            e_bf[0:rows, sc:sc + 1], s_sb[0:rows, sc:sc + 1],
                            AF.Exp, scale=1.0 / S,
                            accum_out=acc[0:rows, sc:sc + 1])
                    for sc in range(SC):
                        rows = 128 if sc < SC - 1 else s_last
                        nc.tensor.matmul(
                            psA[0:1, COL_S:COL_S + 1], ones_col[0:rows, :],
                            acc[0:rows, sc:sc + 1],
                            start=False, stop=(sc == SC - 1),
                            skip_group_check=True)
                # q = e @ M2 (SM-scaled fp8) -> psB cols 8:16
                for mc in range(HC):
                    for tc_i in range(TC):
                        rows = 128 if tc_i < TC - 1 else s_last
                        nc.tensor.matmul(
                            psB[:, HC + mc:HC + mc + 1],
                            m2_sb[0:rows, tc_i, mc * 128:(mc + 1) * 128],
                            e_bf[0:rows, tc_i:tc_i + 1],
                            start=(tc_i == 0), stop=(tc_i == TC - 1),
                            skip_group_check=True)
                rs_bf = work.tile([1, 1], BF, tag="rsbf")
                with nc.allow_low_precision(
                        reason="rs is rounded to bf16 for the broadcast "
                               "matmul either way"):
                    nc.vector.reciprocal(rs_bf[:], psA[0:1, COL_S:COL_S + 1])
                # broadcast rs/SM to all partitions (psB col 16)
                nc.tensor.matmul(psB[:, 2 * HC:2 * HC + 1], sm_row[:], rs_bf[:],
                                 start=True, stop=True, skip_group_check=True)
                rs_col = work.tile([128, 1], F32, tag="rscol")
                nc.vector.tensor_copy(rs_col[:], psB[:, 2 * HC:2 * HC + 1])
                q1 = work.tile([128, HC], F32, tag="q1")
                nc.vector.tensor_scalar(out=q1[:], in0=psB[:, HC:2 * HC],
                                        scalar1=rs_col[:], scalar2=None,
                                        op0=ALU.mult)
                q2 = work.tile([128, HC], F32, tag="q2")
                nc.vector.tensor_tensor(out=q2[:], in0=q1[:],
                                        in1=pc_sb[:, 0:HC, t], op=ALU.add)
                comb_bf = work.tile([128, HC], BF, tag="comb")
                nc.scalar.activation(comb_bf[:], q2[:], AF.Relu)

                # --- dec_Wih @ comb: rz into psA, n into psB cols 8:16 ---
                for c in range(RZC):
                    for kc in range(HC):
                        nc.tensor.matmul(
                            psA[:, RZ0 + c:RZ0 + c + 1],
                            wihd[:, kc, c * 128:(c + 1) * 128],
                            comb_bf[:, kc:kc + 1],
                            start=False, stop=(kc == HC - 1),
                            skip_group_check=True)
                for c in range(HC):
                    gc = RZC + c
                    for kc in range(HC):
                        nc.tensor.matmul(
                            psB[:, HC + c:HC + c + 1],
                            wihd[:, kc, gc * 128:(gc + 1) * 128],
                            comb_bf[:, kc:kc + 1],
                            start=(kc == 0), stop=(kc == HC - 1),
                            skip_group_check=True)

                # --- gates --- (one fused [128,16] bias add covers the
                # n-gate h-part (u) and x-part (xn): psB cols 0:16 against
                # the concatenated [bhn_d | bxn_d] bias)
                uxn = work.tile([128, RZC], F32, tag="uxn")
                nc.vector.tensor_tensor(out=uxn[:], in0=psB[:, 0:RZC],
                                        in1=bhnxn_d[:], op=ALU.add)
                gru_tail(psA[:, RZ0:RZ0 + RZC], uxn[:, 0:HC], brz_d[:],
                         uxn[:, HC:RZC], hsrc, h_decT[:, 0:HC, t])

            decpB_cm.__exit__(None, None, None)
            decpA_cm.__exit__(None, None, None)
            decw_cm.__exit__(None, None, None)

            # ----------------- projection + log_softmax -----------------
            projw_cm = tc.tile_pool(name="projw", bufs=1)
            projw = projw_cm.__enter__()
            logit16 = projw.tile([128, TC, VS], F16, tag="logit16")
            sacc = projw.tile([128, TC, VC], F32, tag="sacc")
            s_loc = projw.tile([128, TC], F32, tag="sloc")
            logz = projw.tile([128, TC], F32, tag="logz")
            nc.vector.memset(sacc[:], 0.0)

            def w_slice(j):
                v0 = j * NV
                nv = min(NV, VS - v0)
                if v0 < VA:
                    return owa, v0, nv
                return owb, v0 - VA, nv

            pj1_cm = tc.tile_pool(name="pj1", bufs=4, space="PSUM")
            pj1 = pj1_cm.__enter__()
            scr_cm = tc.tile_pool(name="scr", bufs=3)
            scr = scr_cm.__enter__()

            # single matmul pass: psum -> fp16 logits in SBUF -> exp-sum
            for m in range(TC):
                rows = 128 if m < TC - 1 else s_last
                for j in range(VC):
                    src, off, nv = w_slice(j)
                    v0 = j * NV
                    ps = pj1.tile([128, NV], F32, tag="pj1")
                    nc.tensor.matmul(ps[0:rows, 0:nv], ones_bf[0:1, 0:rows],
                                     outb_sb[0:1, v0:v0 + nv],
                                     start=True, stop=False,
                                     skip_group_check=True)
                    for kc in range(HC):
                        nc.tensor.matmul(
                            ps[0:rows, 0:nv],
                            h_decT[:, kc, m * 128:m * 128 + rows],
                            src[:, kc, off:off + nv],
                            start=False, stop=(kc == HC - 1),
                            skip_group_check=True)
                    lg = logit16[0:rows, m, v0:v0 + nv]
                    nc.vector.tensor_copy(lg, ps[0:rows, 0:nv])
                    escr = scr.tile([128, NV], BF, tag="escr")
                    nc.scalar.activation(
                        escr[0:rows, 0:nv],
                        lg, AF.Exp, accum_out=sacc[0:rows, m, j:j + 1])
            nc.vector.reduce_sum(s_loc[:], sacc[:], axis=mybir.AxisListType.X)

            with tc.tile_pool(name="dram", bufs=1, space="DRAM") as dram:
                ib = dram.tile([128, TC], F32)
                ob = dram.tile([128, TC], F32)
                nc.gpsimd.dma_start(ib[:], s_loc[:])
                nc.gpsimd.collective_compute(
                    "AllReduce", ALU.add,
                    replica_groups=[list(range(n_cores))],
                    ins=[ib.opt()], outs=[ob.opt()])
                s_tot = projw.tile([128, TC], F32, tag="stot")
                nc.sync.dma_start(s_tot[:], ob[:])
                nc.scalar.activation(logz[:], s_tot[:], AF.Ln)

                for m in range(TC):
                    rows = 128 if m < TC - 1 else s_last
                    for j in range(VC):
                        v0 = j * NV
                        nv = min(NV, VS - v0)
                        ot = scr.tile([128, NV], F32, tag="oscr")
                        nc.vector.tensor_scalar(
                            out=ot[0:rows, 0:nv],
                            in0=logit16[0:rows, m, v0:v0 + nv],
                            scalar1=logz[0:rows, m:m + 1], scalar2=None,
                            op0=ALU.subtract)
                        nc.sync.dma_start(
                            out_d[m * 128:m * 128 + rows, v0:v0 + nv],
                            ot[0:rows, 0:nv])

            if debug_outs:
                nc.sync.dma_start(dbg_enc[:],
                                  enc_outsT[:].rearrange("p a b -> p (a b)"))
                nc.sync.dma_start(dbg_hdec[:],
                                  h_decT[:].rearrange("p a b -> p (a b)"))

            scr_cm.__exit__(None, None, None)
            pj1_cm.__exit__(None, None, None)
            projw_cm.__exit__(None, None, None)
            owbp_cm.__exit__(None, None, None)
            oww_cm.__exit__(None, None, None)

        work_cm.__exit__(None, None, None)
        cons_cm.__exit__(None, None, None)

    nc.compile()
    return nc


# ---------------------------------------------------------------------------
# host side
# ---------------------------------------------------------------------------

def _tiles(M, dt=BF16):
    """M [rows, H] -> lhsT tile layout [128, HC*rows] (M.T tiled)."""
    rows = M.shape[0]
    return np.ascontiguousarray(
        M.T.reshape(HC, 128, rows).transpose(1, 0, 2).reshape(128, HC * rows)
    ).astype(dt)


def _cols(v):
    """v [C*128] -> [128, C] fp32 column layout."""
    C = v.shape[0] // 128
    return np.ascontiguousarray(v.reshape(C, 128).T).astype(np.float32)


_PROG_CACHE = {}


def _get_program(T, VS, VA, reps=1):
    key = (T, VS, VA, reps)
    if key not in _PROG_CACHE:
        _PROG_CACHE[key] = build_program(T, VS, VA, reps=reps)
    return _PROG_CACHE[key]


def prepare_inputs(inputs, T, VS, VA):
    f32 = np.float32
    inp = np.asarray(inputs["input_seq"], f32)[:, 0, :]      # [T, H]
    target = np.asarray(inputs["target"]).astype(np.int64)[:, 0]
    emb_dec = np.asarray(inputs["emb_dec"], f32)
    toks = np.concatenate([[SOS], target[:-1]])
    emb_seq = emb_dec[toks]                                   # [T, H]

    attn_W = np.asarray(inputs["attn_W"], f32)
    A1, A2 = attn_W[:, :H], attn_W[:, H:]
    comb_W = np.asarray(inputs["comb_W"], f32)
    C1, C2 = comb_W[:, :H], comb_W[:, H:]
    enc_bih = np.asarray(inputs["enc_bih"], f32)
    enc_bhh = np.asarray(inputs["enc_bhh"], f32)
    dec_bih = np.asarray(inputs["dec_bih"], f32)
    dec_bhh = np.asarray(inputs["dec_bhh"], f32)
    attn_b = np.asarray(inputs["attn_b"], f32)
    comb_b = np.asarray(inputs["comb_b"], f32)
    out_W = np.asarray(inputs["out_W"], f32)
    out_b = np.asarray(inputs["out_b"], f32)

    SC = _cdiv(T, 128)
    stk = np.concatenate([np.asarray(inputs["dec_Whh"], f32), A2], axis=0)

    ve = np.concatenate([(enc_bih + enc_bhh)[:2 * H], enc_bih[2 * H:]])
    attn_b_pad = np.zeros(SC * 128, f32)
    attn_b_pad[:T] = attn_b

    shared = {
        # fp8 S-scaled recurrence weights
        "whhe_t": _tiles(S * np.asarray(inputs["enc_Whh"], f32), FP8),
        "wihd_t": _tiles(S * np.asarray(inputs["dec_Wih"], f32), FP8),
        "wstk_t": _tiles(S * stk, FP8),
        # bf16 batched-phase weights (S folded in where the consumer is
        # S-scaled; exact: S, SM are powers of two)
        "wihe_t": _tiles(S * np.asarray(inputs["enc_Wih"], f32)),
        "c1t": _tiles(C1),
        "c2t": _tiles(SM * C2),
        "a1t": _tiles(S * A1),
        "embt": _tiles(emb_seq),
        "inpt": _tiles(inp),
        "bias_e": _cols(S * ve),
        "bhh_n_e": _cols(S * enc_bhh[2 * H:]),
        "biasd_rz": _cols(S * (dec_bih + dec_bhh)[:2 * H]),
        "biasd_hnxn": np.concatenate(
            [_cols(S * dec_bhh[2 * H:]), _cols(S * dec_bih[2 * H:])], axis=1),
        "bias_a": _cols(S * attn_b_pad),
        "bias_c": _cols(comb_b),
    }

    V = out_W.shape[0]
    in_maps = []
    for c in range(N_CORES):
        vlo = c * VS
        vhi = min(V, vlo + VS)
        Wsh = np.zeros((VS, H), f32)
        bsh = np.full(VS, -1e30, f32)
        if vhi > vlo:
            Wsh[:vhi - vlo] = out_W[vlo:vhi]
            bsh[:vhi - vlo] = out_b[vlo:vhi]
        wt = _tiles(Wsh)                    # [128, HC*VS]
        wt3 = wt.reshape(128, HC, VS)
        m = dict(shared)
        m["out_wta"] = np.ascontiguousarray(wt3[:, :, :VA]).reshape(128, HC * VA)
        m["out_wtb"] = np.ascontiguousarray(wt3[:, :, VA:]).reshape(
            128, HC * (VS - VA))
        m["out_bb"] = bsh.astype(BF16)[None, :]
        in_maps.append(m)
    return in_maps


def run(inputs, T=256, VS=None, VA=None, trace=False):
    V = np.asarray(inputs["out_W"]).shape[0]
    if VS is None:
        VS = _cdiv(_cdiv(V, N_CORES), NV) * NV   # 6656 for V=50257
    if VA is None:
        VA = max(NV, (VS // (2 * NV)) * NV)
    nc = _get_program(T, VS, VA)
    in_maps = prepare_inputs(inputs, T, VS, VA)
    res = run_bass_kernel_spmd(nc, in_maps, core_ids=list(range(N_CORES)),
                               trace=trace)
    parts = []
    for c in range(N_CORES):
        vlo = c * VS
        vhi = min(V, vlo + VS)
        if vhi <= vlo:
            continue
        parts.append(res.results[c]["out"][:, :vhi - vlo])
    full = np.concatenate(parts, axis=1).astype(np.float32)
    return full.reshape(T, 1, V), res


def kernel(**inputs):
    out, _ = run(inputs, T=256)
    return out
